# revision 8
# baseline (speedup 1.0000x reference)
"""Trainium2 Bass kernel for nn_DecoderRNN (embedding lookup + single-layer LSTM).

Problem (hardcoded): B=64, T=32, V=32000, E=512, H=1024.
  emb    = one_hot(captions) @ W_embed.T + b_embed        (= row gather of W_embed.T)
  inputs = concat([features, emb], time)                   [B, 33, E]
  out    = LSTM(inputs, h0, c0)                            [B, 33, H]

Strategy:
  - Host-side layout prep only (transposes / bf16 casts / column permutation of
    weights, index flattening, bias folding). No FLOPs of the reference op on
    the host beyond tiny bias vectors.
  - Embedding lookup: indirect-DMA row gather from host-pre-transposed
    W_embed.T [V, E] (bf16).
  - 2-way data parallel: core c handles batch half (c % 2) (32 rows). All 8
    cores run the identical program (cores 2..7 process duplicate halves;
    their outputs are ignored) - SPMD, no collectives.
  - x-projection X @ W_ih.T precomputed for all 33 steps (PE, bf16), staged
    to DRAM in the folded layout below.
  - Recurrence: gates_h = h @ W_hh.T as 4-way column-tiled packed matmuls.
    Folded layout: PSUM [128, 1024], partition 32*g + b, column 256*q + c
      == gate q (order i,f,o,g~) of batch row b, hidden column 256*g + c.
    Each PE column-group g accumulates all 8 K-chunks for its quarter of H
    (weight columns host-permuted so each group's 1024 columns are
    contiguous) -> full 128x128 PE utilization at M=32, and every
    elementwise op runs on all 128 partitions at base partition 0.
  - Elementwise split across ACT (sigmoid/tanh), DVE, GpSimd; h re-transposed
    each step on the PE (row-group-tiled transposes) for the next step's
    stationary operand.
"""

import os
import sys

sys.path.insert(0, "/opt/trn_rl_repo")

import numpy as np
import ml_dtypes

B, T, V, E, H = 64, 32, 32000, 512, 1024
NT = T + 1          # 33 time steps
B2 = B // 2         # 32 rows per core
KC = H // 128       # 8 k-chunks of the recurrent contraction
EC = E // 128       # 4 k-chunks of the input contraction
G4 = 4 * H          # 4096 gate columns
HQ = H // 4         # 256 = hidden quarter
N_CORES = 8
NTOK = B2 * NT      # 1056 rows of X per core (t-major)

# gate order in the folded column layout: i, f, o, g~  (sigmoid on cols 0:768)
QOFF = [0, H, 3 * H, 2 * H]

_BF = ml_dtypes.bfloat16

_compiled = None


def _fold_cols(w):
    """Permute gate columns [4096] so that group g's slice is contiguous:
    newcol(g, q, c) = 1024*g + 256*q + c  <-  oldcol = QOFF[q] + 256*g + c."""
    idx = np.empty(G4, np.int64)
    for g in range(4):
        for q in range(4):
            base = 1024 * g + HQ * q
            idx[base:base + HQ] = QOFF[q] + HQ * g + np.arange(HQ)
    return w[..., idx]


def _build_nc():
    import concourse.mybir as mybir
    import concourse.tile as tile
    from concourse import bacc
    from concourse.masks import make_identity
    import concourse.bass as bass

    # debug bisection: "full" | "xonly" | "nogather" | "norec"
    phase_mode = os.environ.get("KERNEL_PHASES", "full")
    do_gather = phase_mode not in ("nogather",)
    do_rec = phase_mode not in ("xonly", "norec")

    bf = mybir.dt.bfloat16
    f32 = mybir.dt.float32
    Sig = mybir.ActivationFunctionType.Sigmoid
    Tanh = mybir.ActivationFunctionType.Tanh

    nc = bacc.Bacc(None, target_bir_lowering=False, debug=False)

    idx_d = nc.dram_tensor("idx", [128, 8], mybir.dt.int32, kind="ExternalInput")
    wembT_d = nc.dram_tensor("wembT", [V, E], bf, kind="ExternalInput")
    featT_d = nc.dram_tensor("featT", [E, B2], bf, kind="ExternalInput")
    wihT_d = nc.dram_tensor("wihT", [E, G4], bf, kind="ExternalInput")
    whhT_d = nc.dram_tensor("whhT", [H, G4], bf, kind="ExternalInput")
    biasA_d = nc.dram_tensor("biasA", [128, G4], bf, kind="ExternalInput")
    biasB_d = nc.dram_tensor("biasB", [128, G4], bf, kind="ExternalInput")
    h0T_d = nc.dram_tensor("h0T", [H, B2], bf, kind="ExternalInput")
    c0_d = nc.dram_tensor("c0", [128, HQ], f32, kind="ExternalInput")
    id32_d = nc.dram_tensor("id32", [128, B2], f32, kind="ExternalInput")
    hs_d = nc.dram_tensor("hs", [NT, B2, H], f32, kind="ExternalOutput")

    with tile.TileContext(nc) as tc:
        with tc.tile_pool(name="const", bufs=1) as cp, \
             tc.tile_pool(name="dram", bufs=1, space="DRAM") as dp:
            whh_sb = cp.tile([128, KC * G4], bf)
            for k in range(KC):
                nc.sync.dma_start(whh_sb[:, k * G4:(k + 1) * G4],
                                  whhT_d[k * 128:(k + 1) * 128, :])
            ident_bf = cp.tile([128, 128], bf)
            make_identity(nc, ident_bf[:])
            id32_sb = cp.tile([128, B2], f32)
            nc.sync.dma_start(id32_sb[:], id32_d[:])
            idx_sb = cp.tile([128, 8], mybir.dt.int32)
            nc.sync.dma_start(idx_sb[:], idx_d[:])
            gxd = dp.tile([NT, 128, H], f32)

            # ---------------- phase X: gather + transpose + x-projection ----
            with tc.tile_pool(name="xw", bufs=1) as xp, \
                 tc.tile_pool(name="xg", bufs=2) as xgp, \
                 tc.tile_pool(name="ptx", bufs=2, space="PSUM") as ptxp, \
                 tc.tile_pool(name="px", bufs=1, space="PSUM") as pxp, \
                 tc.tile_pool(name="sx", bufs=2) as sxp:
                wih_sb = xp.tile([128, EC * G4], bf)
                for e in range(EC):
                    nc.sync.dma_start(wih_sb[:, e * G4:(e + 1) * G4],
                                      wihT_d[e * 128:(e + 1) * 128, :])
                biasA_sb = xp.tile([128, G4], bf)
                nc.sync.dma_start(biasA_sb[:], biasA_d[:])
                biasB_sb = xp.tile([128, G4], bf)
                nc.sync.dma_start(biasB_sb[:], biasB_d[:])

                # X.T, e-chunk major: col e*NTOK + r  (r = X row, t-major)
                xT = xp.tile([128, EC * NTOK], bf)
                for e in range(EC):
                    nc.sync.dma_start(xT[:, e * NTOK:e * NTOK + B2],
                                      featT_d[e * 128:(e + 1) * 128, :])
                # gather 8 x 128 embedding rows, then PE-transpose into xT
                for j in range(8):
                    xg = xgp.tile([128, E], bf, tag="xg")
                    if do_gather:
                        nc.gpsimd.indirect_dma_start(
                            out=xg[:],
                            out_offset=None,
                            in_=wembT_d[:],
                            in_offset=bass.IndirectOffsetOnAxis(
                                ap=idx_sb[:, j:j + 1], axis=0),
                        )
                    else:
                        nc.gpsimd.memset(xg[:], 0.0)
                    for e in range(EC):
                        tp = ptxp.tile([128, 128], bf, tag="tp")
                        nc.tensor.transpose(tp[:], xg[:, e * 128:(e + 1) * 128],
                                            ident_bf[:])
                        nc.vector.tensor_copy(
                            xT[:, e * NTOK + B2 + 128 * j:
                               e * NTOK + B2 + 128 * (j + 1)], tp[:])

                # x-projection, token blocks of 128 (t-major; block m = t 4m..4m+3)
                for m in range(9):
                    mw = 128 if m < 8 else B2
                    for half in range(2):
                        px = pxp.tile([128, 2048], f32, tag="px")
                        for e in range(EC):
                            for n in range(4):
                                nc.tensor.matmul(
                                    px[:mw, 512 * n:512 * (n + 1)],
                                    xT[:, e * NTOK + 128 * m:
                                       e * NTOK + 128 * m + mw],
                                    wih_sb[:, e * G4 + 2048 * half + 512 * n:
                                           e * G4 + 2048 * half + 512 * n + 512],
                                    start=(e == 0), stop=(e == EC - 1),
                                )
                        bias_sb = biasA_sb if m == 0 else biasB_sb
                        sx = sxp.tile([128, 2048], f32, tag="sx")
                        nc.vector.tensor_add(
                            sx[:mw], px[:mw],
                            bias_sb[0:mw, 2048 * half:2048 * (half + 1)])
                        # fold to DRAM: sx cols are already (g, q, c)-permuted,
                        # so group slices are contiguous: half covers groups
                        # 2*half, 2*half+1.
                        for i in range(4 if m < 8 else 1):
                            for gi in range(2):
                                g = 2 * half + gi
                                nc.sync.dma_start(
                                    gxd[4 * m + i, 32 * g:32 * g + 32, :],
                                    sx[32 * i:32 * i + 32,
                                       1024 * gi:1024 * (gi + 1)])

            if not do_rec:
                with tc.tile_pool(name="dbg", bufs=1) as dbgp:
                    dbg = dbgp.tile([128, H], f32)
                    nc.sync.dma_start(dbg[:], gxd[0, :, :])
                    for q in range(4):
                        nc.sync.dma_start(hs_d[q, :, :],
                                          dbg[32 * q:32 * (q + 1), :])
                nc_done = True
            # ---------------- recurrence ----------------
            if do_rec:
              with tc.tile_pool(name="rgx", bufs=3) as gxp, \
                 tc.tile_pool(name="rwork", bufs=2) as rp, \
                 tc.tile_pool(name="pg", bufs=2, space="PSUM") as pgp, \
                 tc.tile_pool(name="pt", bufs=2, space="PSUM") as ptp:

                hT_cur = rp.tile([128, KC * B2], bf, tag="hT")
                for k in range(KC):
                    nc.sync.dma_start(hT_cur[:, B2 * k:B2 * (k + 1)],
                                      h0T_d[128 * k:128 * (k + 1), :])
                c_cur = rp.tile([128, HQ], f32, tag="c")
                nc.sync.dma_start(c_cur[:], c0_d[:])

                gx_tiles = {}

                def fetch_gx(t):
                    if t >= NT:
                        return
                    g = gxp.tile([128, H], f32, tag="gx")
                    nc.sync.dma_start(g[:], gxd[t, :, :])
                    gx_tiles[t] = g

                fetch_gx(0)
                fetch_gx(1)

                for t in range(NT):
                    fetch_gx(t + 2)
                    gx = gx_tiles.pop(t)

                    psg = pgp.tile([128, H], f32, tag="psg")
                    for n in range(2):
                        for k in range(KC):
                            for g in range(4):
                                co = k * G4 + 1024 * g + 512 * n
                                nc.tensor.matmul(
                                    psg[32 * g:32 * (g + 1),
                                        512 * n:512 * (n + 1)],
                                    hT_cur[:, B2 * k:B2 * k + 32],
                                    whh_sb[:, co:co + 512],
                                    start=(k == 0), stop=(k == KC - 1),
                                    tile_position=(0, 32 * g),
                                    skip_group_check=True,
                                )

                    pa = rp.tile([128, H], f32, tag="pa")
                    act = rp.tile([128, H], f32, tag="act")
                    c_new = rp.tile([128, HQ], f32, tag="c")
                    t1 = rp.tile([128, HQ], f32, tag="t1")
                    t2 = rp.tile([128, HQ], f32, tag="t2")
                    tct = rp.tile([128, HQ], f32, tag="tct")
                    h = rp.tile([128, HQ], f32, tag="h")

                    for n in range(2):
                        s = slice(512 * n, 512 * (n + 1))
                        nc.vector.tensor_add(pa[:, s], psg[:, s], gx[:, s])
                    # cols: [0:256]=i [256:512]=f [512:768]=o [768:1024]=g~
                    nc.scalar.activation(act[:, 0:768], pa[:, 0:768], Sig)
                    nc.scalar.activation(act[:, 768:1024], pa[:, 768:1024], Tanh)
                    # c' = sig(f)*c + sig(i)*tanh(g~);  h = sig(o)*tanh(c')
                    nc.vector.tensor_mul(t1[:], act[:, 256:512], c_cur[:])
                    nc.gpsimd.tensor_mul(t2[:], act[:, 0:256], act[:, 768:1024])
                    nc.vector.tensor_add(c_new[:], t1[:], t2[:])
                    nc.scalar.activation(tct[:], c_new[:], Tanh)
                    nc.vector.tensor_mul(h[:], act[:, 512:768], tct[:])
                    # h folded: partition 32g+b, col c -> h[b, 256g + c]
                    for g in range(4):
                        nc.sync.dma_start(hs_d[t, :, HQ * g:HQ * (g + 1)],
                                          h[32 * g:32 * (g + 1), :])

                    if t < NT - 1:
                        # hT via selector matmuls: out[:, 32k:+32] =
                        #   h_bf[:, 128*(k%2):+128].T @ I[:, 32*(k//2):+32]
                        # (full-partition operands; avoids partial-partition
                        # transpose-mode ops)
                        h_bf = rp.tile([128, HQ], bf, tag="hbf")
                        nc.vector.tensor_copy(h_bf[:], h[:])
                        pt = ptp.tile([128, KC * B2], f32, tag="pt")
                        for k in range(KC):
                            nc.tensor.matmul(
                                pt[:, 32 * k:32 * (k + 1)],
                                h_bf[:, 128 * (k % 2):128 * (k % 2 + 1)],
                                ident_bf[:, 32 * (k // 2):32 * (k // 2) + 32],
                                start=True, stop=True,
                                skip_group_check=True,
                            )
                        hT_next = rp.tile([128, KC * B2], bf, tag="hT")
                        nc.vector.tensor_copy(hT_next[:], pt[:])
                        hT_cur = hT_next
                    c_cur = c_new

    nc.finalize()
    return nc


def _get_compiled():
    global _compiled
    if _compiled is None:
        _compiled = _build_nc()
    return _compiled


def _fold_rows(x):
    """[32, 1024] -> [128, 256]: out[32g+b, c] = x[b, 256g+c]."""
    return np.ascontiguousarray(
        x.reshape(B2, 4, HQ).transpose(1, 0, 2).reshape(128, HQ))


def _prep_core_inputs(half, features, captions, W_embedT_bf, wihT_bf,
                      whhT_bf, biasA, biasB, h0, c0, id32):
    sl = slice(half * B2, (half + 1) * B2)
    feat = features[sl]                       # [32, 512]
    cap = captions[sl]                        # [32, 32]
    # token indices, (t, b)-major for t=1..32: tok[(t-1)*32 + b] = cap[b, t-1]
    tok = np.ascontiguousarray(cap.T).reshape(-1).astype(np.int32)   # [1024]
    idx = np.ascontiguousarray(tok.reshape(8, 128).T)                # [128, 8]
    return dict(
        idx=idx,
        wembT=W_embedT_bf,
        featT=np.ascontiguousarray(feat.T).astype(_BF),
        wihT=wihT_bf,
        whhT=whhT_bf,
        biasA=biasA,
        biasB=biasB,
        h0T=np.ascontiguousarray(h0[sl].T).astype(_BF),
        c0=_fold_rows(np.ascontiguousarray(c0[sl]).astype(np.float32)),
        id32=id32,
    )


def kernel(features, captions, W_embed, b_embed, w_ih, w_hh, b_ih, b_hh, h0, c0):
    from concourse.bass_utils import run_bass_kernel_spmd

    features = np.asarray(features, dtype=np.float32)
    captions = np.asarray(captions, dtype=np.int32)
    W_embed = np.asarray(W_embed, dtype=np.float32)
    b_embed = np.asarray(b_embed, dtype=np.float32)
    w_ih = np.asarray(w_ih, dtype=np.float32)
    w_hh = np.asarray(w_hh, dtype=np.float32)
    b_ih = np.asarray(b_ih, dtype=np.float32)
    b_hh = np.asarray(b_hh, dtype=np.float32)
    h0 = np.asarray(h0, dtype=np.float32)
    c0 = np.asarray(c0, dtype=np.float32)

    # host layout prep
    W_embedT_bf = np.ascontiguousarray(W_embed.T).astype(_BF)        # [V, E]
    wihT_bf = np.ascontiguousarray(_fold_cols(w_ih.T)).astype(_BF)   # [E, 4H]
    whhT_bf = np.ascontiguousarray(_fold_cols(w_hh.T)).astype(_BF)   # [H, 4H]
    bias0 = _fold_cols((b_ih + b_hh).astype(np.float32))             # t = 0
    bias1 = bias0 + _fold_cols((b_embed @ w_ih.T).astype(np.float32))
    biasA = np.empty((128, G4), np.float32)
    biasA[0:32] = bias0
    biasA[32:128] = bias1
    biasB = np.broadcast_to(bias1, (128, G4)).copy()
    biasA = biasA.astype(_BF)
    biasB = biasB.astype(_BF)
    # replicated 32x32 identity (for row-group-tiled PE transposes of h)
    id32 = np.zeros((128, B2), np.float32)
    for p in range(128):
        id32[p, p % 32] = 1.0

    nc = _get_compiled()
    in_maps = []
    for c in range(N_CORES):
        in_maps.append(_prep_core_inputs(c % 2, features, captions, W_embedT_bf,
                                         wihT_bf, whhT_bf, biasA, biasB,
                                         h0, c0, id32))
    res = run_bass_kernel_spmd(nc, in_maps, list(range(N_CORES)),
                               trace=bool(int(os.environ.get("KERNEL_TRACE", "0"))))
    kernel.last_results = res

    out = np.empty((B, NT, H), np.float32)
    for half in range(2):
        hs = res.results[half]["hs"]          # [33, 32, 1024]
        out[half * B2:(half + 1) * B2] = np.transpose(hs, (1, 0, 2))
    return out


# revision 9
# speedup vs baseline: 1.2184x; 1.2184x over previous
"""Trainium2 Bass kernel for nn_DecoderRNN (embedding lookup + single-layer LSTM).

Problem (hardcoded): B=64, T=32, V=32000, E=512, H=1024.
  emb    = one_hot(captions) @ W_embed.T + b_embed        (= row gather of W_embed.T)
  inputs = concat([features, emb], time)                   [B, 33, E]
  out    = LSTM(inputs, h0, c0)                            [B, 33, H]

Strategy:
  - Host-side layout prep only (transposes / bf16 casts / column permutation of
    weights, index flattening, bias folding). No FLOPs of the reference op on
    the host beyond tiny bias vectors.
  - Embedding lookup: indirect-DMA row gather from host-pre-transposed
    W_embed.T [V, E] (bf16).
  - 2-way data parallel: core c handles batch half (c % 2) (32 rows). All 8
    cores run the identical program (cores 2..7 process duplicate halves;
    their outputs are ignored) - SPMD, no collectives.
  - Recurrence: gates_h = h @ W_hh.T as 4-way column-tiled packed matmuls.
    Folded layout: PSUM [128, 1024], partition 32*g + b, column
    512*n + 128*q + c == gate q (order i,f,o,g~) of batch row b, hidden
    column 256*g + 128*n + c.  Each PE column-group g accumulates all 8
    K-chunks for its quarter of H (weight columns host-permuted so each
    group's slice is contiguous) -> full 128x128 PE utilization at M=32.
    The interleave by n lets the whole cell-update chain pipeline in two
    512-column halves, each half containing all four gates.
  - gates_x = X @ W_ih.T precomputed on the PE (token blocks of 128), staged
    to DRAM as bf16 in the same folded layout, and added into PSUM with
    identity-selector matmuls (start=False accumulation).
  - x-projection emission is interleaved into the recurrence loop to keep the
    PE busy during elementwise windows (HAM stays at full clock).
  - h is re-transposed each step via identity-selector matmuls (full-partition
    operands; partial-partition transpose-mode ops crash the HW).
"""

import os
import sys

sys.path.insert(0, "/opt/trn_rl_repo")

import numpy as np
import ml_dtypes

B, T, V, E, H = 64, 32, 32000, 512, 1024
NT = T + 1          # 33 time steps
B2 = B // 2         # 32 rows per core
KC = H // 128       # 8 k-chunks of the recurrent contraction
EC = E // 128       # 4 k-chunks of the input contraction
G4 = 4 * H          # 4096 gate columns
HQ = H // 4         # 256 = hidden quarter
N_CORES = 8
NTOK = B2 * NT      # 1056 rows of X per core (t-major)

# gate order in the folded column layout: i, f, o, g~
QOFF = [0, H, 3 * H, 2 * H]

_BF = ml_dtypes.bfloat16

_compiled = None


def _fold_cols(w):
    """Permute gate columns [4096]:
    newcol(g, n, q, c128) = 1024g + 512n + 128q + c  <-
        oldcol = QOFF[q] + 256g + 128n + c."""
    idx = np.empty(G4, np.int64)
    for g in range(4):
        for n in range(2):
            for q in range(4):
                base = 1024 * g + 512 * n + 128 * q
                idx[base:base + 128] = QOFF[q] + HQ * g + 128 * n + np.arange(128)
    return w[..., idx]


def _build_nc():
    import concourse.mybir as mybir
    import concourse.tile as tile
    from concourse import bacc
    from concourse.masks import make_identity
    import concourse.bass as bass

    bf = mybir.dt.bfloat16
    f32 = mybir.dt.float32
    Sig = mybir.ActivationFunctionType.Sigmoid
    Tanh = mybir.ActivationFunctionType.Tanh

    nc = bacc.Bacc(None, target_bir_lowering=False, debug=False)

    idx_d = nc.dram_tensor("idx", [128, 8], mybir.dt.int32, kind="ExternalInput")
    wembT_d = nc.dram_tensor("wembT", [V, E], bf, kind="ExternalInput")
    featT_d = nc.dram_tensor("featT", [E, B2], bf, kind="ExternalInput")
    wihT_d = nc.dram_tensor("wihT", [E, G4], bf, kind="ExternalInput")
    whhT_d = nc.dram_tensor("whhT", [H, G4], bf, kind="ExternalInput")
    biasA_d = nc.dram_tensor("biasA", [128, G4], bf, kind="ExternalInput")
    biasB_d = nc.dram_tensor("biasB", [128, G4], bf, kind="ExternalInput")
    h0T_d = nc.dram_tensor("h0T", [H, B2], bf, kind="ExternalInput")
    c0_d = nc.dram_tensor("c0", [128, HQ], f32, kind="ExternalInput")
    hs_d = nc.dram_tensor("hs", [NT, B2, H], f32, kind="ExternalOutput")

    with tile.TileContext(nc) as tc:
        with tc.tile_pool(name="const", bufs=1) as cp, \
             tc.tile_pool(name="dram", bufs=1, space="DRAM") as dp:
            whh_sb = cp.tile([128, KC * G4], bf)
            for k in range(KC):
                nc.sync.dma_start(whh_sb[:, k * G4:(k + 1) * G4],
                                  whhT_d[k * 128:(k + 1) * 128, :])
            ident_bf = cp.tile([128, 128], bf)
            make_identity(nc, ident_bf[:])
            idx_sb = cp.tile([128, 8], mybir.dt.int32)
            nc.sync.dma_start(idx_sb[:], idx_d[:])
            gxd = dp.tile([NT, 128, H], bf)

            wih_sb = cp.tile([128, EC * G4], bf)
            for e in range(EC):
                nc.sync.dma_start(wih_sb[:, e * G4:(e + 1) * G4],
                                  wihT_d[e * 128:(e + 1) * 128, :])
            biasA_sb = cp.tile([128, G4], bf)
            nc.sync.dma_start(biasA_sb[:], biasA_d[:])
            biasB_sb = cp.tile([128, G4], bf)
            nc.sync.dma_start(biasB_sb[:], biasB_d[:])
            # X.T, e-chunk major: col e*NTOK + r  (r = X row, t-major)
            xT = cp.tile([128, EC * NTOK], bf)

            # ---------------- gather + transpose ----------------
            with tc.tile_pool(name="xg", bufs=2) as xgp, \
                 tc.tile_pool(name="ptx", bufs=2, space="PSUM") as ptxp:
                for e in range(EC):
                    nc.sync.dma_start(xT[:, e * NTOK:e * NTOK + B2],
                                      featT_d[e * 128:(e + 1) * 128, :])
                for j in range(8):
                    xg = xgp.tile([128, E], bf, tag="xg")
                    nc.gpsimd.indirect_dma_start(
                        out=xg[:],
                        out_offset=None,
                        in_=wembT_d[:],
                        in_offset=bass.IndirectOffsetOnAxis(
                            ap=idx_sb[:, j:j + 1], axis=0),
                    )
                    for e in range(EC):
                        tp = ptxp.tile([128, 128], bf, tag="tp")
                        nc.tensor.transpose(tp[:], xg[:, e * 128:(e + 1) * 128],
                                            ident_bf[:])
                        nc.vector.tensor_copy(
                            xT[:, e * NTOK + B2 + 128 * j:
                               e * NTOK + B2 + 128 * (j + 1)], tp[:])

            # ---------------- x-projection + recurrence ----------------
            with tc.tile_pool(name="px", bufs=1, space="PSUM") as pxp, \
                 tc.tile_pool(name="sx", bufs=2) as sxp, \
                 tc.tile_pool(name="rgx", bufs=3) as gxp, \
                 tc.tile_pool(name="rwork", bufs=2) as rp, \
                 tc.tile_pool(name="pg", bufs=1, space="PSUM") as pgp, \
                 tc.tile_pool(name="pt", bufs=2, space="PSUM") as ptp:

                def xproj_unit(m, half):
                    """One (token-block, 2048-column half) of the x-projection.
                    Writes gxd[4m .. 4m+3] group-pairs (2*half, 2*half+1)."""
                    mw = 128 if m < 8 else B2
                    px = pxp.tile([128, 2048], f32, tag="px")
                    for e in range(EC):
                        for n in range(4):
                            nc.tensor.matmul(
                                px[:mw, 512 * n:512 * (n + 1)],
                                xT[:, e * NTOK + 128 * m:
                                   e * NTOK + 128 * m + mw],
                                wih_sb[:, e * G4 + 2048 * half + 512 * n:
                                       e * G4 + 2048 * half + 512 * n + 512],
                                start=(e == 0), stop=(e == EC - 1),
                            )
                    bias_sb = biasA_sb if m == 0 else biasB_sb
                    sx = sxp.tile([128, 2048], bf, tag="sx")
                    nc.vector.tensor_add(
                        sx[:mw], px[:mw],
                        bias_sb[0:mw, 2048 * half:2048 * (half + 1)])
                    for i in range(4 if m < 8 else 1):
                        for gi in range(2):
                            g = 2 * half + gi
                            nc.sync.dma_start(
                                gxd[4 * m + i, 32 * g:32 * g + 32, :],
                                sx[32 * i:32 * i + 32,
                                   1024 * gi:1024 * (gi + 1)])

                # prologue: blocks 0-1 cover t = 0..7
                for m in range(2):
                    for half in range(2):
                        xproj_unit(m, half)
                xp_units = [(m, half) for m in range(2, 9) for half in range(2)]

                hT_cur = rp.tile([128, KC * B2], bf, tag="hT")
                for k in range(KC):
                    nc.sync.dma_start(hT_cur[:, B2 * k:B2 * (k + 1)],
                                      h0T_d[128 * k:128 * (k + 1), :])
                c_cur = rp.tile([128, HQ], f32, tag="c")
                nc.sync.dma_start(c_cur[:], c0_d[:])

                gx_tiles = {}

                def fetch_gx(t):
                    if t >= NT:
                        return
                    g = gxp.tile([128, H], bf, tag="gx")
                    nc.sync.dma_start(g[:], gxd[t, :, :])
                    gx_tiles[t] = g

                fetch_gx(0)
                fetch_gx(1)

                for t in range(NT):
                    fetch_gx(t + 2)
                    gx = gx_tiles.pop(t)

                    psg = pgp.tile([128, H], f32, tag="psg")
                    for n in range(2):
                        for k in range(KC):
                            for g in range(4):
                                co = k * G4 + 1024 * g + 512 * n
                                nc.tensor.matmul(
                                    psg[32 * g:32 * (g + 1),
                                        512 * n:512 * (n + 1)],
                                    hT_cur[:, B2 * k:B2 * k + 32],
                                    whh_sb[:, co:co + 512],
                                    start=(k == 0), stop=False,
                                    tile_position=(0, 32 * g),
                                    skip_group_check=True,
                                )
                        # += gates_x via identity-selector (final accumulate)
                        for g in range(4):
                            nc.tensor.matmul(
                                psg[32 * g:32 * (g + 1), 512 * n:512 * (n + 1)],
                                ident_bf[:, 32 * g:32 * (g + 1)],
                                gx[:, 512 * n:512 * (n + 1)],
                                start=False, stop=True,
                                tile_position=(0, 32 * g),
                                skip_group_check=True,
                            )

                    # keep the PE warm during the elementwise window
                    if xp_units and t % 2 == 0:
                        xproj_unit(*xp_units.pop(0))

                    act = rp.tile([128, H], f32, tag="act")
                    c_new = rp.tile([128, HQ], f32, tag="c")
                    t1 = rp.tile([128, HQ], f32, tag="t1")
                    t2 = rp.tile([128, HQ], f32, tag="t2")
                    tct = rp.tile([128, HQ], f32, tag="tct")
                    h = rp.tile([128, HQ], f32, tag="h")
                    h_bf = rp.tile([128, HQ], bf, tag="hbf")
                    pt = ptp.tile([128, KC * B2], f32, tag="pt")
                    hT_next = rp.tile([128, KC * B2], bf, tag="hT")

                    for n in range(2):
                        a = 512 * n          # half base: [i f o g~] x 128
                        q = slice(128 * n, 128 * (n + 1))  # c/h quarter slice
                        nc.scalar.activation(act[:, a:a + 384],
                                             psg[:, a:a + 384], Sig)
                        nc.scalar.activation(act[:, a + 384:a + 512],
                                             psg[:, a + 384:a + 512], Tanh)
                        nc.vector.tensor_mul(t1[:, q], act[:, a + 128:a + 256],
                                             c_cur[:, q])
                        nc.gpsimd.tensor_mul(t2[:, q], act[:, a:a + 128],
                                             act[:, a + 384:a + 512])
                        nc.vector.tensor_add(c_new[:, q], t1[:, q], t2[:, q])
                        nc.scalar.activation(tct[:, q], c_new[:, q], Tanh)
                        nc.vector.tensor_mul(h[:, q], act[:, a + 256:a + 384],
                                             tct[:, q])
                        nc.gpsimd.tensor_mul(h_bf[:, q], act[:, a + 256:a + 384],
                                             tct[:, q])
                        if t < NT - 1:
                            # hT chunks k with k%2 == n via selector matmuls
                            for g in range(4):
                                k = 2 * g + n
                                nc.tensor.matmul(
                                    pt[:, 32 * k:32 * (k + 1)],
                                    h_bf[:, q],
                                    ident_bf[:, 32 * g:32 * (g + 1)],
                                    start=True, stop=True,
                                    skip_group_check=True,
                                )
                            # copy this half's chunks (strided: every other k)
                            src = pt[:].rearrange("p (k c) -> p k c", c=B2)
                            dst = hT_next[:].rearrange("p (k c) -> p k c", c=B2)
                            nc.vector.tensor_copy(dst[:, n::2, :], src[:, n::2, :])

                    # h folded: partition 32g+b, col c -> h[b, 256g + c]
                    for g in range(4):
                        nc.sync.dma_start(hs_d[t, :, HQ * g:HQ * (g + 1)],
                                          h[32 * g:32 * (g + 1), :])

                    if t < NT - 1:
                        hT_cur = hT_next
                    c_cur = c_new

    nc.finalize()
    return nc


def _get_compiled():
    global _compiled
    if _compiled is None:
        _compiled = _build_nc()
    return _compiled


def _fold_rows(x):
    """[32, 1024] -> [128, 256]: out[32g+b, c] = x[b, 256g+c]."""
    return np.ascontiguousarray(
        x.reshape(B2, 4, HQ).transpose(1, 0, 2).reshape(128, HQ))


def _prep_core_inputs(half, features, captions, W_embedT_bf, wihT_bf,
                      whhT_bf, biasA, biasB, h0, c0):
    sl = slice(half * B2, (half + 1) * B2)
    feat = features[sl]                       # [32, 512]
    cap = captions[sl]                        # [32, 32]
    # token indices, (t, b)-major for t=1..32: tok[(t-1)*32 + b] = cap[b, t-1]
    tok = np.ascontiguousarray(cap.T).reshape(-1).astype(np.int32)   # [1024]
    idx = np.ascontiguousarray(tok.reshape(8, 128).T)                # [128, 8]
    return dict(
        idx=idx,
        wembT=W_embedT_bf,
        featT=np.ascontiguousarray(feat.T).astype(_BF),
        wihT=wihT_bf,
        whhT=whhT_bf,
        biasA=biasA,
        biasB=biasB,
        h0T=np.ascontiguousarray(h0[sl].T).astype(_BF),
        c0=_fold_rows(np.ascontiguousarray(c0[sl]).astype(np.float32)),
    )


def kernel(features, captions, W_embed, b_embed, w_ih, w_hh, b_ih, b_hh, h0, c0):
    from concourse.bass_utils import run_bass_kernel_spmd

    features = np.asarray(features, dtype=np.float32)
    captions = np.asarray(captions, dtype=np.int32)
    W_embed = np.asarray(W_embed, dtype=np.float32)
    b_embed = np.asarray(b_embed, dtype=np.float32)
    w_ih = np.asarray(w_ih, dtype=np.float32)
    w_hh = np.asarray(w_hh, dtype=np.float32)
    b_ih = np.asarray(b_ih, dtype=np.float32)
    b_hh = np.asarray(b_hh, dtype=np.float32)
    h0 = np.asarray(h0, dtype=np.float32)
    c0 = np.asarray(c0, dtype=np.float32)

    # host layout prep
    W_embedT_bf = np.ascontiguousarray(W_embed.T).astype(_BF)        # [V, E]
    wihT_bf = np.ascontiguousarray(_fold_cols(w_ih.T)).astype(_BF)   # [E, 4H]
    whhT_bf = np.ascontiguousarray(_fold_cols(w_hh.T)).astype(_BF)   # [H, 4H]
    bias0 = _fold_cols((b_ih + b_hh).astype(np.float32))             # t = 0
    bias1 = bias0 + _fold_cols((b_embed @ w_ih.T).astype(np.float32))
    biasA = np.empty((128, G4), np.float32)
    biasA[0:32] = bias0
    biasA[32:128] = bias1
    biasB = np.broadcast_to(bias1, (128, G4)).copy()
    biasA = biasA.astype(_BF)
    biasB = biasB.astype(_BF)

    nc = _get_compiled()
    in_maps = []
    for c in range(N_CORES):
        in_maps.append(_prep_core_inputs(c % 2, features, captions, W_embedT_bf,
                                         wihT_bf, whhT_bf, biasA, biasB,
                                         h0, c0))
    res = run_bass_kernel_spmd(nc, in_maps, list(range(N_CORES)),
                               trace=bool(int(os.environ.get("KERNEL_TRACE", "0"))))
    kernel.last_results = res

    out = np.empty((B, NT, H), np.float32)
    for half in range(2):
        hs = res.results[half]["hs"]          # [33, 32, 1024]
        out[half * B2:(half + 1) * B2] = np.transpose(hs, (1, 0, 2))
    return out


# revision 11
# speedup vs baseline: 1.4720x; 1.2082x over previous
"""Trainium2 Bass kernel for nn_DecoderRNN (embedding lookup + single-layer LSTM).

Problem (hardcoded): B=64, T=32, V=32000, E=512, H=1024.
  emb    = one_hot(captions) @ W_embed.T + b_embed        (= row gather of W_embed.T)
  inputs = concat([features, emb], time)                   [B, 33, E]
  out    = LSTM(inputs, h0, c0)                            [B, 33, H]

Strategy:
  - Host-side layout prep only (transposes / bf16 casts / column permutation of
    weights, index flattening, bias folding).
  - Embedding lookup: indirect-DMA row gather from host-pre-transposed
    W_embed.T [V, E] (bf16).
  - 2-way data parallel: core c handles batch half (c % 2). All 8 cores run
    the identical program (cores 2..7 duplicate; outputs ignored). No
    collectives.
  - Recurrence: gates_h = h @ W_hh.T as 4-way column-tiled packed matmuls.
    Folded layout: PSUM [128, 1024], partition 32*g + b, column
    512*n + 128*q + c == gate q (order i,f,o,g~) of batch row b, hidden
    column 256*g + 128*n + c.  Each PE column-group g accumulates all 8
    K-chunks of its quarter of H (weight columns host-permuted so each
    group's slice is contiguous) -> full 128x128 PE utilization at M=32.
    The n-interleave lets the cell update pipeline in two 512-column halves,
    each containing all four gates at full 128 partitions.
  - gates_x = X @ W_ih.T precomputed on the PE (token blocks of 128, bias
    folded in as an extra contraction row), staged to DRAM as bf16 in the
    folded layout, added into PSUM with identity-selector matmuls.
  - x-projection quarter-units are interleaved into the recurrence loop to
    keep the PE warm (HAM) during elementwise windows.
  - h.T for the next step via ONE full-identity matmul per half (the folded
    layout makes out[:, 32g:+32] exactly h.T chunk 2g+n), fp32 operands.
"""

import os
import sys

sys.path.insert(0, "/opt/trn_rl_repo")

import numpy as np
import ml_dtypes

B, T, V, E, H = 64, 32, 32000, 512, 1024
NT = T + 1          # 33 time steps
B2 = B // 2         # 32 rows per core
KC = H // 128       # 8 k-chunks of the recurrent contraction
EC = E // 128       # 4 k-chunks of the input contraction
G4 = 4 * H          # 4096 gate columns
HQ = H // 4         # 256 = hidden quarter
N_CORES = 8
NTOK = B2 * NT      # 1056 rows of X per core (t-major)

# gate order in the folded column layout: i, f, o, g~
QOFF = [0, H, 3 * H, 2 * H]

_BF = ml_dtypes.bfloat16

_compiled = None


def _fold_cols(w):
    """Permute gate columns [4096]:
    newcol(g, n, q, c128) = 1024g + 512n + 128q + c  <-
        oldcol = QOFF[q] + 256g + 128n + c."""
    idx = np.empty(G4, np.int64)
    for g in range(4):
        for n in range(2):
            for q in range(4):
                base = 1024 * g + 512 * n + 128 * q
                idx[base:base + 128] = QOFF[q] + HQ * g + 128 * n + np.arange(128)
    return w[..., idx]


def _build_nc():
    import concourse.mybir as mybir
    import concourse.tile as tile
    from concourse import bacc
    from concourse.masks import make_identity
    import concourse.bass as bass

    bf = mybir.dt.bfloat16
    f32 = mybir.dt.float32
    Sig = mybir.ActivationFunctionType.Sigmoid
    Tanh = mybir.ActivationFunctionType.Tanh

    nc = bacc.Bacc(None, target_bir_lowering=False, debug=False)

    idx_d = nc.dram_tensor("idx", [128, 8], mybir.dt.int32, kind="ExternalInput")
    wembT_d = nc.dram_tensor("wembT", [V, E], bf, kind="ExternalInput")
    featT_d = nc.dram_tensor("featT", [E, B2], bf, kind="ExternalInput")
    wihT_d = nc.dram_tensor("wihT", [E, G4], bf, kind="ExternalInput")
    whhT_d = nc.dram_tensor("whhT", [H, G4], bf, kind="ExternalInput")
    brow_d = nc.dram_tensor("brow", [2, G4], bf, kind="ExternalInput")
    bsel_d = nc.dram_tensor("bsel", [2, 128], bf, kind="ExternalInput")
    h0T_d = nc.dram_tensor("h0T", [H, B2], bf, kind="ExternalInput")
    c0_d = nc.dram_tensor("c0", [128, HQ], f32, kind="ExternalInput")
    hs_d = nc.dram_tensor("hs", [NT, B2, H], f32, kind="ExternalOutput")

    with tile.TileContext(nc) as tc:
        with tc.tile_pool(name="const", bufs=1) as cp, \
             tc.tile_pool(name="dram", bufs=1, space="DRAM") as dp:
            whh_sb = cp.tile([128, KC * G4], bf)
            for k in range(KC):
                nc.sync.dma_start(whh_sb[:, k * G4:(k + 1) * G4],
                                  whhT_d[k * 128:(k + 1) * 128, :])
            ident_f = cp.tile([128, 128], f32)
            make_identity(nc, ident_f[:])
            ident_bf = cp.tile([128, 128], bf)
            nc.vector.tensor_copy(ident_bf[:], ident_f[:])
            idx_sb = cp.tile([128, 8], mybir.dt.int32)
            nc.sync.dma_start(idx_sb[:], idx_d[:])
            gxd = dp.tile([NT, 128, H], bf)

            wih_sb = cp.tile([128, EC * G4], bf)
            for e in range(EC):
                nc.sync.dma_start(wih_sb[:, e * G4:(e + 1) * G4],
                                  wihT_d[e * 128:(e + 1) * 128, :])
            brow_sb = cp.tile([2, G4], bf)
            nc.sync.dma_start(brow_sb[:], brow_d[:])
            bsel_sb = cp.tile([2, 128], bf)
            nc.sync.dma_start(bsel_sb[:], bsel_d[:])
            # X.T, e-chunk major: col e*NTOK + r  (r = X row, t-major)
            xT = cp.tile([128, EC * NTOK], bf)

            # ---------------- gather + transpose ----------------
            with tc.tile_pool(name="xg", bufs=2) as xgp, \
                 tc.tile_pool(name="ptx", bufs=2, space="PSUM") as ptxp:
                for e in range(EC):
                    nc.sync.dma_start(xT[:, e * NTOK:e * NTOK + B2],
                                      featT_d[e * 128:(e + 1) * 128, :])
                for j in range(8):
                    xg = xgp.tile([128, E], bf, tag="xg")
                    nc.gpsimd.indirect_dma_start(
                        out=xg[:],
                        out_offset=None,
                        in_=wembT_d[:],
                        in_offset=bass.IndirectOffsetOnAxis(
                            ap=idx_sb[:, j:j + 1], axis=0),
                    )
                    for e in range(EC):
                        tp = ptxp.tile([128, 128], bf, tag="tp")
                        nc.tensor.transpose(tp[:], xg[:, e * 128:(e + 1) * 128],
                                            ident_bf[:])
                        nc.vector.tensor_copy(
                            xT[:, e * NTOK + B2 + 128 * j:
                               e * NTOK + B2 + 128 * (j + 1)], tp[:])

            # ---------------- x-projection + recurrence ----------------
            with tc.tile_pool(name="px", bufs=2, space="PSUM") as pxp, \
                 tc.tile_pool(name="sx", bufs=2) as sxp, \
                 tc.tile_pool(name="rgx", bufs=3) as gxp, \
                 tc.tile_pool(name="rwork", bufs=2) as rp, \
                 tc.tile_pool(name="pg", bufs=1, space="PSUM") as pgp, \
                 tc.tile_pool(name="pt", bufs=2, space="PSUM") as ptp:

                def xproj_unit_mm(m, g):
                    """Matmuls of one (token-block m, folded group g) quarter
                    of the x-projection: px [128, 1024] = gates_x cols
                    1024g:+1024 for X rows 128m:+mw (+ bias via extra rows)."""
                    mw = 128 if m < 8 else B2
                    px = pxp.tile([128, 1024], f32, tag="px")
                    for e in range(EC):
                        for nn in range(2):
                            nc.tensor.matmul(
                                px[:mw, 512 * nn:512 * (nn + 1)],
                                xT[:, e * NTOK + 128 * m:
                                   e * NTOK + 128 * m + mw],
                                wih_sb[:, e * G4 + 1024 * g + 512 * nn:
                                       e * G4 + 1024 * g + 512 * nn + 512],
                                start=(e == 0), stop=False,
                            )
                    nb = 2 if m == 0 else 1
                    for nn in range(2):
                        nc.tensor.matmul(
                            px[:mw, 512 * nn:512 * (nn + 1)],
                            bsel_sb[0:nb, 0:mw],
                            brow_sb[0:nb, 1024 * g + 512 * nn:
                                    1024 * g + 512 * nn + 512],
                            start=False, stop=True,
                        )
                    return px, mw

                def xproj_unit_tail(m, g, px, mw):
                    """Cast + stage one quarter to DRAM (split DVE/GpSimd)."""
                    sx = sxp.tile([128, 1024], bf, tag="sx")
                    nc.vector.tensor_copy(sx[:mw, 0:512], px[:mw, 0:512])
                    nc.scalar.copy(sx[:mw, 512:1024], px[:mw, 512:1024])
                    for i in range(4 if m < 8 else 1):
                        nc.sync.dma_start(
                            gxd[4 * m + i, 32 * g:32 * g + 32, :],
                            sx[32 * i:32 * i + 32, :])

                def xproj_unit(m, g):
                    px, mw = xproj_unit_mm(m, g)
                    xproj_unit_tail(m, g, px, mw)

                # prologue: blocks 0-1 (t = 0..7)
                for m in range(2):
                    for g in range(4):
                        xproj_unit(m, g)
                xp_units = [(m, g) for m in range(2, 9) for g in range(4)]

                hT_cur = rp.tile([128, KC * B2], bf, tag="hT")
                for k in range(KC):
                    nc.sync.dma_start(hT_cur[:, B2 * k:B2 * (k + 1)],
                                      h0T_d[128 * k:128 * (k + 1), :])
                c_cur = rp.tile([128, HQ], f32, tag="c")
                nc.sync.dma_start(c_cur[:], c0_d[:])

                gx_tiles = {}

                def fetch_gx(t):
                    if t >= NT:
                        return
                    g = gxp.tile([128, H], bf, tag="gx")
                    nc.sync.dma_start(g[:], gxd[t, :, :])
                    gx_tiles[t] = g

                fetch_gx(0)
                fetch_gx(1)

                KORDER = [0, 2, 4, 6, 1, 3, 5, 7]   # even h.T chunks first

                for t in range(NT):
                    fetch_gx(t + 2)
                    gx = gx_tiles.pop(t)

                    psg = pgp.tile([128, H], f32, tag="psg")
                    for n in range(2):
                        for ki, k in enumerate(KORDER):
                            for g in range(4):
                                co = k * G4 + 1024 * g + 512 * n
                                nc.tensor.matmul(
                                    psg[32 * g:32 * (g + 1),
                                        512 * n:512 * (n + 1)],
                                    hT_cur[:, B2 * k:B2 * k + 32],
                                    whh_sb[:, co:co + 512],
                                    start=(ki == 0), stop=False,
                                    tile_position=(0, 32 * g),
                                    skip_group_check=True,
                                )
                        # += gates_x via identity-selector (final accumulate)
                        for g in range(4):
                            nc.tensor.matmul(
                                psg[32 * g:32 * (g + 1), 512 * n:512 * (n + 1)],
                                ident_bf[:, 32 * g:32 * (g + 1)],
                                gx[:, 512 * n:512 * (n + 1)],
                                start=False, stop=True,
                                tile_position=(0, 32 * g),
                                skip_group_check=True,
                            )

                    # keep the PE warm during the elementwise window
                    xp = xp_units.pop(0) if xp_units else None
                    if xp is not None:
                        xp_px = xproj_unit_mm(*xp)

                    act = rp.tile([128, H], f32, tag="act")
                    c_new = rp.tile([128, HQ], f32, tag="c")
                    t1 = rp.tile([128, HQ], f32, tag="t1")
                    t2 = rp.tile([128, HQ], f32, tag="t2")
                    tct = rp.tile([128, HQ], f32, tag="tct")
                    h = rp.tile([128, HQ], f32, tag="h")
                    hT_next = rp.tile([128, KC * B2], bf, tag="hT")

                    for n in range(2):
                        a = 512 * n          # half base: [i f o g~] x 128
                        q = slice(128 * n, 128 * (n + 1))  # c/h quarter cols
                        eng1 = nc.vector if n == 0 else nc.gpsimd
                        eng2 = nc.gpsimd if n == 0 else nc.vector
                        nc.scalar.activation(act[:, a:a + 384],
                                             psg[:, a:a + 384], Sig)
                        nc.scalar.activation(act[:, a + 384:a + 512],
                                             psg[:, a + 384:a + 512], Tanh)
                        eng1.tensor_mul(t1[:, q], act[:, a + 128:a + 256],
                                        c_cur[:, q])
                        eng2.tensor_mul(t2[:, q], act[:, a:a + 128],
                                        act[:, a + 384:a + 512])
                        eng1.tensor_add(c_new[:, q], t1[:, q], t2[:, q])
                        nc.scalar.activation(tct[:, q], c_new[:, q], Tanh)
                        nc.vector.tensor_mul(h[:, q], act[:, a + 256:a + 384],
                                             tct[:, q])
                        if t < NT - 1:
                            # ONE matmul: pt[:, 32g:+32] == h.T chunk 2g+n
                            pt = ptp.tile([128, 128], f32, tag="pt")
                            nc.tensor.matmul(
                                pt[:], h[:, q], ident_f[:],
                                start=True, stop=True,
                                skip_group_check=True,
                            )
                            dst = hT_next[:].rearrange("p (k c) -> p k c", c=B2)
                            src = pt[:].rearrange("p (g c) -> p g c", c=B2)
                            nc.vector.tensor_copy(dst[:, n::2, :], src[:])

                    # h folded: partition 32g+b, col c -> h[b, 256g + c]
                    for g in range(4):
                        nc.sync.dma_start(hs_d[t, :, HQ * g:HQ * (g + 1)],
                                          h[32 * g:32 * (g + 1), :])

                    if xp is not None:
                        xproj_unit_tail(*xp, *xp_px)

                    if t < NT - 1:
                        hT_cur = hT_next
                    c_cur = c_new

    nc.finalize()
    return nc


def _get_compiled():
    global _compiled
    if _compiled is None:
        _compiled = _build_nc()
    return _compiled


def _fold_rows(x):
    """[32, 1024] -> [128, 256]: out[32g+b, c] = x[b, 256g+c]."""
    return np.ascontiguousarray(
        x.reshape(B2, 4, HQ).transpose(1, 0, 2).reshape(128, HQ))


def _prep_core_inputs(half, features, captions, W_embedT_bf, wihT_bf,
                      whhT_bf, brow, bsel, h0, c0):
    sl = slice(half * B2, (half + 1) * B2)
    feat = features[sl]                       # [32, 512]
    cap = captions[sl]                        # [32, 32]
    # token indices, (t, b)-major for t=1..32: tok[(t-1)*32 + b] = cap[b, t-1]
    tok = np.ascontiguousarray(cap.T).reshape(-1).astype(np.int32)   # [1024]
    idx = np.ascontiguousarray(tok.reshape(8, 128).T)                # [128, 8]
    return dict(
        idx=idx,
        wembT=W_embedT_bf,
        featT=np.ascontiguousarray(feat.T).astype(_BF),
        wihT=wihT_bf,
        whhT=whhT_bf,
        brow=brow,
        bsel=bsel,
        h0T=np.ascontiguousarray(h0[sl].T).astype(_BF),
        c0=_fold_rows(np.ascontiguousarray(c0[sl]).astype(np.float32)),
    )


def kernel(features, captions, W_embed, b_embed, w_ih, w_hh, b_ih, b_hh, h0, c0):
    from concourse.bass_utils import run_bass_kernel_spmd

    features = np.asarray(features, dtype=np.float32)
    captions = np.asarray(captions, dtype=np.int32)
    W_embed = np.asarray(W_embed, dtype=np.float32)
    b_embed = np.asarray(b_embed, dtype=np.float32)
    w_ih = np.asarray(w_ih, dtype=np.float32)
    w_hh = np.asarray(w_hh, dtype=np.float32)
    b_ih = np.asarray(b_ih, dtype=np.float32)
    b_hh = np.asarray(b_hh, dtype=np.float32)
    h0 = np.asarray(h0, dtype=np.float32)
    c0 = np.asarray(c0, dtype=np.float32)

    # host layout prep
    W_embedT_bf = np.ascontiguousarray(W_embed.T).astype(_BF)        # [V, E]
    wihT_bf = np.ascontiguousarray(_fold_cols(w_ih.T)).astype(_BF)   # [E, 4H]
    whhT_bf = np.ascontiguousarray(_fold_cols(w_hh.T)).astype(_BF)   # [H, 4H]
    bias0 = _fold_cols((b_ih + b_hh).astype(np.float32))             # t = 0
    bias1 = bias0 + _fold_cols((b_embed @ w_ih.T).astype(np.float32))
    # bias via extra contraction rows: row0 (all tokens) = bias1,
    # row1 (t=0 tokens only, selected by bsel row 1) = bias0 - bias1
    brow = np.stack([bias1, bias0 - bias1]).astype(_BF)              # [2, 4096]
    bsel = np.zeros((2, 128), np.float32)
    bsel[0, :] = 1.0
    bsel[1, 0:32] = 1.0                       # block 0 rows 0:32 are t=0
    bsel = bsel.astype(_BF)

    nc = _get_compiled()
    in_maps = []
    for c in range(N_CORES):
        in_maps.append(_prep_core_inputs(c % 2, features, captions, W_embedT_bf,
                                         wihT_bf, whhT_bf, brow, bsel,
                                         h0, c0))
    res = run_bass_kernel_spmd(nc, in_maps, list(range(N_CORES)),
                               trace=bool(int(os.environ.get("KERNEL_TRACE", "0"))))
    kernel.last_results = res

    out = np.empty((B, NT, H), np.float32)
    for half in range(2):
        hs = res.results[half]["hs"]          # [33, 32, 1024]
        out[half * B2:(half + 1) * B2] = np.transpose(hs, (1, 0, 2))
    return out


# revision 17
# speedup vs baseline: 1.6159x; 1.0977x over previous
"""Trainium2 Bass kernel for nn_DecoderRNN (embedding lookup + single-layer LSTM).

Problem (hardcoded): B=64, T=32, V=32000, E=512, H=1024.
  emb    = one_hot(captions) @ W_embed.T + b_embed        (= row gather of W_embed.T)
  inputs = concat([features, emb], time)                   [B, 33, E]
  out    = LSTM(inputs, h0, c0)                            [B, 33, H]

Strategy:
  - Host-side layout prep only (transposes / bf16 casts / column permutation of
    weights, index flattening, bias folding).
  - Embedding lookup: indirect-DMA row gather from host-pre-transposed
    W_embed.T [V, E] (bf16).
  - 2-way data parallel: core c handles batch half (c % 2). All 8 cores run
    the identical program (cores 2..7 duplicate; outputs ignored). No
    collectives.
  - Recurrence: gates_h = h @ W_hh.T as 4-way column-tiled packed matmuls.
    Folded layout: PSUM [128, 1024], partition 32*g + b, column
    512*n + 128*q + c == gate q (order i,f,o,g~) of batch row b, hidden
    column 256*g + 128*n + c.  Each PE column-group g accumulates all 8
    K-chunks of its quarter of H (weight columns host-permuted so each
    group's slice is contiguous) -> full 128x128 PE utilization at M=32.
    The n-interleave lets the cell update pipeline in two 512-column halves,
    each containing all four gates at full 128 partitions.
  - gates_x = X @ W_ih.T precomputed on the PE (token blocks of 128, bias
    folded in as an extra contraction row), staged to DRAM as bf16 in the
    folded layout, added into PSUM with identity-selector matmuls.
  - x-projection quarter-units are interleaved into the recurrence loop to
    keep the PE warm (HAM) during elementwise windows.
  - h.T for the next step via ONE full-identity matmul per half (the folded
    layout makes out[:, 32g:+32] exactly h.T chunk 2g+n), fp32 operands.
"""

import os
import sys

sys.path.insert(0, "/opt/trn_rl_repo")

import numpy as np
import ml_dtypes

B, T, V, E, H = 64, 32, 32000, 512, 1024
NT = T + 1          # 33 time steps
B2 = B // 2         # 32 rows per core
KC = H // 128       # 8 k-chunks of the recurrent contraction
EC = E // 128       # 4 k-chunks of the input contraction
G4 = 4 * H          # 4096 gate columns
HQ = H // 4         # 256 = hidden quarter
N_CORES = 8
NTOK = B2 * NT      # 1056 rows of X per core (t-major)

# gate order in the folded column layout: i, f, o, g~
QOFF = [0, H, 3 * H, 2 * H]

_BF = ml_dtypes.bfloat16

_compiled = None


def _fold_cols(w):
    """Permute gate columns [4096]:
    newcol(g, n, q, c128) = 1024g + 512n + 128q + c  <-
        oldcol = QOFF[q] + 256g + 128n + c."""
    idx = np.empty(G4, np.int64)
    for g in range(4):
        for n in range(2):
            for q in range(4):
                base = 1024 * g + 512 * n + 128 * q
                idx[base:base + 128] = QOFF[q] + HQ * g + 128 * n + np.arange(128)
    return w[..., idx]


def _build_nc():
    import concourse.mybir as mybir
    import concourse.tile as tile
    from concourse import bacc
    from concourse.masks import make_identity
    import concourse.bass as bass

    bf = mybir.dt.bfloat16
    f32 = mybir.dt.float32
    Sig = mybir.ActivationFunctionType.Sigmoid
    Tanh = mybir.ActivationFunctionType.Tanh

    nc = bacc.Bacc(None, target_bir_lowering=False, debug=False)

    idx_d = nc.dram_tensor("idx", [128, 8], mybir.dt.int32, kind="ExternalInput")
    wembT_d = nc.dram_tensor("wembT", [V, E], bf, kind="ExternalInput")
    featT_d = nc.dram_tensor("featT", [E, B2], bf, kind="ExternalInput")
    wihT_d = nc.dram_tensor("wihT", [E, G4], bf, kind="ExternalInput")
    whhT_d = nc.dram_tensor("whhT", [H, G4], bf, kind="ExternalInput")
    brow_d = nc.dram_tensor("brow", [2, G4], bf, kind="ExternalInput")
    bsel_d = nc.dram_tensor("bsel", [2, 128], bf, kind="ExternalInput")
    h0T_d = nc.dram_tensor("h0T", [H, B2], bf, kind="ExternalInput")
    c0_d = nc.dram_tensor("c0", [2, 128, 128], f32, kind="ExternalInput")
    hs_d = nc.dram_tensor("hs", [NT, B2, H], f32, kind="ExternalOutput")

    with tile.TileContext(nc) as tc:
        with tc.tile_pool(name="const", bufs=1) as cp, \
             tc.tile_pool(name="dram", bufs=1, space="DRAM") as dp:
            # wih first: the prologue x-projection needs it right away;
            # whh is only needed once the recurrence starts.
            wih_sb = cp.tile([128, EC * G4], bf)
            for e in range(EC):
                nc.sync.dma_start(wih_sb[:, e * G4:(e + 1) * G4],
                                  wihT_d[e * 128:(e + 1) * 128, :])
            brow_sb = cp.tile([2, G4], bf)
            nc.sync.dma_start(brow_sb[:], brow_d[:])
            bsel_sb = cp.tile([2, 128], bf)
            nc.sync.dma_start(bsel_sb[:], bsel_d[:])
            ident_f = cp.tile([128, 128], f32)
            make_identity(nc, ident_f[:])
            ident_bf = cp.tile([128, 128], bf)
            nc.vector.tensor_copy(ident_bf[:], ident_f[:])
            idx_sb = cp.tile([128, 8], mybir.dt.int32)
            nc.sync.dma_start(idx_sb[:], idx_d[:])
            gxd = dp.tile([NT, 128, H], bf)

            whh_sb = cp.tile([128, KC * G4], bf)
            for k in range(KC):
                nc.sync.dma_start(whh_sb[:, k * G4:(k + 1) * G4],
                                  whhT_d[k * 128:(k + 1) * 128, :])
            # X.T, e-chunk major: col e*NTOK + r  (r = X row, t-major)
            xT = cp.tile([128, EC * NTOK], bf)

            # ---------------- gather + transpose ----------------
            with tc.tile_pool(name="xg", bufs=2) as xgp, \
                 tc.tile_pool(name="ptx", bufs=2, space="PSUM") as ptxp:
                for e in range(EC):
                    nc.sync.dma_start(xT[:, e * NTOK:e * NTOK + B2],
                                      featT_d[e * 128:(e + 1) * 128, :])
                for j in range(8):
                    xg = xgp.tile([128, E], bf, tag="xg")
                    nc.gpsimd.indirect_dma_start(
                        out=xg[:],
                        out_offset=None,
                        in_=wembT_d[:],
                        in_offset=bass.IndirectOffsetOnAxis(
                            ap=idx_sb[:, j:j + 1], axis=0),
                    )
                    for e in range(EC):
                        tp = ptxp.tile([128, 128], bf, tag="tp")
                        nc.tensor.transpose(tp[:], xg[:, e * 128:(e + 1) * 128],
                                            ident_bf[:])
                        nc.vector.tensor_copy(
                            xT[:, e * NTOK + B2 + 128 * j:
                               e * NTOK + B2 + 128 * (j + 1)], tp[:])

            # ---------------- x-projection + recurrence ----------------
            with tc.tile_pool(name="px", bufs=2, space="PSUM") as pxp, \
                 tc.tile_pool(name="sx", bufs=2) as sxp, \
                 tc.tile_pool(name="rgx", bufs=3) as gxp, \
                 tc.tile_pool(name="rwork", bufs=2) as rp, \
                 tc.tile_pool(name="pg", bufs=1, space="PSUM") as pgp, \
                 tc.tile_pool(name="pt", bufs=2, space="PSUM") as ptp:

                def xproj_unit_mm(m, g):
                    """Matmuls of one (token-block m, folded group g) quarter
                    of the x-projection: px [128, 1024] = gates_x cols
                    1024g:+1024 for X rows 128m:+mw (+ bias via extra rows)."""
                    mw = 128 if m < 8 else B2
                    px = pxp.tile([128, 1024], f32, tag="px")
                    for e in range(EC):
                        for nn in range(2):
                            nc.tensor.matmul(
                                px[:mw, 512 * nn:512 * (nn + 1)],
                                xT[:, e * NTOK + 128 * m:
                                   e * NTOK + 128 * m + mw],
                                wih_sb[:, e * G4 + 1024 * g + 512 * nn:
                                       e * G4 + 1024 * g + 512 * nn + 512],
                                start=(e == 0), stop=False,
                            )
                    nb = 2 if m == 0 else 1
                    for nn in range(2):
                        nc.tensor.matmul(
                            px[:mw, 512 * nn:512 * (nn + 1)],
                            bsel_sb[0:nb, 0:mw],
                            brow_sb[0:nb, 1024 * g + 512 * nn:
                                    1024 * g + 512 * nn + 512],
                            start=False, stop=True,
                        )
                    return px, mw

                def xproj_unit_tail(m, g, px, mw):
                    """Cast + stage one quarter to DRAM (split DVE/GpSimd)."""
                    sx = sxp.tile([128, 1024], bf, tag="sx")
                    nc.vector.tensor_copy(sx[:mw, 0:512], px[:mw, 0:512])
                    nc.scalar.copy(sx[:mw, 512:1024], px[:mw, 512:1024])
                    for i in range(4 if m < 8 else 1):
                        nc.sync.dma_start(
                            gxd[4 * m + i, 32 * g:32 * g + 32, :],
                            sx[32 * i:32 * i + 32, :])

                def xproj_unit(m, g):
                    px, mw = xproj_unit_mm(m, g)
                    xproj_unit_tail(m, g, px, mw)

                # prologue: block 0 only (t = 0..3); the rest interleaves
                for g in range(4):
                    xproj_unit(0, g)
                xp_units = [(m, g) for m in range(1, 9) for g in range(4)]

                # h.T held as two tiles: even chunks (n=0) and odd (n=1);
                # chunk k lives at hT[k % 2][:, 32 * (k // 2) : +32]
                hT_cur = [rp.tile([128, 128], bf, tag=f"hT{par}", name=f"hTc{par}")
                          for par in range(2)]
                for k in range(KC):
                    nc.sync.dma_start(
                        hT_cur[k % 2][:, 32 * (k // 2):32 * (k // 2) + 32],
                        h0T_d[128 * k:128 * (k + 1), :])
                # c as two half tiles (quarter-columns 128n..)
                c_cur = [rp.tile([128, 128], f32, tag=f"c{par}", name=f"cc{par}")
                         for par in range(2)]
                for par in range(2):
                    nc.sync.dma_start(c_cur[par][:], c0_d[par, :, :])

                gx_tiles = {}

                def fetch_gx(t):
                    if t >= NT:
                        return
                    g = gxp.tile([128, H], bf, tag="gx")
                    nc.sync.dma_start(g[:], gxd[t, :, :])
                    gx_tiles[t] = g

                fetch_gx(0)
                fetch_gx(1)

                KORDER = [0, 2, 4, 6, 1, 3, 5, 7]   # even h.T chunks first

                for t in range(NT):
                    fetch_gx(t + 2)
                    gx = gx_tiles.pop(t)

                    # two independent PSUM halves so half 0's consumers
                    # release as soon as its own matmuls finish
                    psg = [pgp.tile([128, 512], f32, tag=f"psg{par}", name=f"psg{par}")
                           for par in range(2)]
                    for n in range(2):
                        for ki, k in enumerate(KORDER):
                            for g in range(4):
                                co = k * G4 + 1024 * g + 512 * n
                                nc.tensor.matmul(
                                    psg[n][32 * g:32 * (g + 1), :],
                                    hT_cur[k % 2][:, 32 * (k // 2):
                                                  32 * (k // 2) + 32],
                                    whh_sb[:, co:co + 512],
                                    start=(ki == 0), stop=False,
                                    tile_position=(0, 32 * g),
                                    skip_group_check=True,
                                )
                        # += gates_x via identity-selector (final accumulate)
                        for g in range(4):
                            nc.tensor.matmul(
                                psg[n][32 * g:32 * (g + 1), :],
                                ident_bf[:, 32 * g:32 * (g + 1)],
                                gx[:, 512 * n:512 * (n + 1)],
                                start=False, stop=True,
                                tile_position=(0, 32 * g),
                                skip_group_check=True,
                            )

                    # keep the PE warm during the elementwise window
                    xps = []
                    nxp = 2 if t < 4 else 1
                    for _ in range(nxp):
                        if xp_units:
                            xp = xp_units.pop(0)
                            xps.append((xp, xproj_unit_mm(*xp)))

                    act = rp.tile([128, H], f32, tag="act")
                    tct = rp.tile([128, HQ], f32, tag="tct")
                    t1 = rp.tile([128, HQ], f32, tag="t1")
                    t2 = rp.tile([128, HQ], f32, tag="t2")
                    c_new = [rp.tile([128, 128], f32, tag=f"c{par}", name=f"cn{par}")
                             for par in range(2)]
                    h = [rp.tile([128, 128], f32, tag=f"h{par}", name=f"h{par}")
                         for par in range(2)]
                    hT_next = [rp.tile([128, 128], bf, tag=f"hT{par}", name=f"hTn{par}")
                               for par in range(2)]

                    for n in range(2):
                        a = 512 * n          # half base: [i f o g~] x 128
                        q = slice(128 * n, 128 * (n + 1))  # scratch cols
                        eng1 = nc.vector if n == 0 else nc.gpsimd
                        eng2 = nc.gpsimd if n == 0 else nc.vector
                        nc.scalar.activation(act[:, a:a + 384],
                                             psg[n][:, 0:384], Sig)
                        nc.scalar.activation(act[:, a + 384:a + 512],
                                             psg[n][:, 384:512], Tanh)
                        eng1.tensor_mul(t1[:, q], act[:, a + 128:a + 256],
                                        c_cur[n][:])
                        eng2.tensor_mul(t2[:, q], act[:, a:a + 128],
                                        act[:, a + 384:a + 512])
                        eng1.tensor_add(c_new[n][:], t1[:, q], t2[:, q])
                        nc.scalar.activation(tct[:, q], c_new[n][:], Tanh)
                        nc.vector.tensor_mul(h[n][:], act[:, a + 256:a + 384],
                                             tct[:, q])
                        if t < NT - 1:
                            # ONE matmul: pt[:, 32g:+32] == h.T chunk 2g+n
                            pt = ptp.tile([128, 128], f32, tag="pt")
                            nc.tensor.matmul(
                                pt[:], h[n][:], ident_f[:],
                                start=True, stop=True,
                                skip_group_check=True,
                            )
                            nc.vector.tensor_copy(hT_next[n][:], pt[:])

                    # h folded: partition 32g+b -> h[b, 256g + 128n + c]
                    for g in range(4):
                        for n in range(2):
                            nc.sync.dma_start(
                                hs_d[t, :, HQ * g + 128 * n:
                                     HQ * g + 128 * (n + 1)],
                                h[n][32 * g:32 * (g + 1), :])

                    for xp, xp_px in xps:
                        xproj_unit_tail(*xp, *xp_px)

                    if t < NT - 1:
                        hT_cur = hT_next
                    c_cur = c_new

    nc.finalize()
    return nc


def _get_compiled():
    global _compiled
    if _compiled is None:
        _compiled = _build_nc()
    return _compiled


def _fold_rows(x):
    """[32, 1024] -> [128, 256]: out[32g+b, c] = x[b, 256g+c]."""
    return np.ascontiguousarray(
        x.reshape(B2, 4, HQ).transpose(1, 0, 2).reshape(128, HQ))


def _prep_core_inputs(half, features, captions, W_embedT_bf, wihT_bf,
                      whhT_bf, brow, bsel, h0, c0):
    sl = slice(half * B2, (half + 1) * B2)
    feat = features[sl]                       # [32, 512]
    cap = captions[sl]                        # [32, 32]
    # token indices, (t, b)-major for t=1..32: tok[(t-1)*32 + b] = cap[b, t-1]
    tok = np.ascontiguousarray(cap.T).reshape(-1).astype(np.int32)   # [1024]
    idx = np.ascontiguousarray(tok.reshape(8, 128).T)                # [128, 8]
    return dict(
        idx=idx,
        wembT=W_embedT_bf,
        featT=np.ascontiguousarray(feat.T).astype(_BF),
        wihT=wihT_bf,
        whhT=whhT_bf,
        brow=brow,
        bsel=bsel,
        h0T=np.ascontiguousarray(h0[sl].T).astype(_BF),
        c0=np.ascontiguousarray(
            _fold_rows(np.ascontiguousarray(c0[sl]).astype(np.float32))
            .reshape(128, 2, 128).transpose(1, 0, 2)),
    )


def kernel(features, captions, W_embed, b_embed, w_ih, w_hh, b_ih, b_hh, h0, c0):
    from concourse.bass_utils import run_bass_kernel_spmd

    features = np.asarray(features, dtype=np.float32)
    captions = np.asarray(captions, dtype=np.int32)
    W_embed = np.asarray(W_embed, dtype=np.float32)
    b_embed = np.asarray(b_embed, dtype=np.float32)
    w_ih = np.asarray(w_ih, dtype=np.float32)
    w_hh = np.asarray(w_hh, dtype=np.float32)
    b_ih = np.asarray(b_ih, dtype=np.float32)
    b_hh = np.asarray(b_hh, dtype=np.float32)
    h0 = np.asarray(h0, dtype=np.float32)
    c0 = np.asarray(c0, dtype=np.float32)

    # host layout prep
    W_embedT_bf = np.ascontiguousarray(W_embed.T).astype(_BF)        # [V, E]
    wihT_bf = np.ascontiguousarray(_fold_cols(w_ih.T)).astype(_BF)   # [E, 4H]
    whhT_bf = np.ascontiguousarray(_fold_cols(w_hh.T)).astype(_BF)   # [H, 4H]
    bias0 = _fold_cols((b_ih + b_hh).astype(np.float32))             # t = 0
    bias1 = bias0 + _fold_cols((b_embed @ w_ih.T).astype(np.float32))
    # bias via extra contraction rows: row0 (all tokens) = bias1,
    # row1 (t=0 tokens only, selected by bsel row 1) = bias0 - bias1
    brow = np.stack([bias1, bias0 - bias1]).astype(_BF)              # [2, 4096]
    bsel = np.zeros((2, 128), np.float32)
    bsel[0, :] = 1.0
    bsel[1, 0:32] = 1.0                       # block 0 rows 0:32 are t=0
    bsel = bsel.astype(_BF)

    nc = _get_compiled()
    in_maps = []
    for c in range(N_CORES):
        in_maps.append(_prep_core_inputs(c % 2, features, captions, W_embedT_bf,
                                         wihT_bf, whhT_bf, brow, bsel,
                                         h0, c0))
    res = run_bass_kernel_spmd(nc, in_maps, list(range(N_CORES)),
                               trace=bool(int(os.environ.get("KERNEL_TRACE", "0"))))
    kernel.last_results = res

    out = np.empty((B, NT, H), np.float32)
    for half in range(2):
        hs = res.results[half]["hs"]          # [33, 32, 1024]
        out[half * B2:(half + 1) * B2] = np.transpose(hs, (1, 0, 2))
    return out
